# revision 1
# baseline (speedup 1.0000x reference)
"""BSplineSynapse Trainium2 kernel (8-core tensor-parallel over out_features).

Math: reference computes, with t = clip(|x|, 0, 1), s = 1 - t:
    w(t) = cp0*s^3 + 3*cp1*s^2*t + 3*cp2*s*t^2 + cp3*t^3   (per (o, i))
    out[b, o] = sum_i w[o, i](t[b, i]) * x[b, i]

Rewritten in the monomial basis of t with the factor-3 folded into the
moving side (g1 = 3 t x, g2 = 3 t^2 x, g3 = 3 t^3 x):
    out = x @ w0^T + g1 @ A^T + g2 @ D2^T + g3 @ D3^T
    A  = w1 - w0
    D2 = w0 - 2*w1 + w2
    D3 = w3/3 - w0/3 + w1 - w2

Engine assignment (fast path, valid when 0 <= x <= 1 so t == x):
  - ScalarE: g1 = 3x^2 = Square(sqrt3*x); g3 = 3x^4 = Square(g1/sqrt3)
  - VectorE: g2 = x * g1; A = w1 - w0; the PSUM->SBUF casts of D2/D3
  - TensorE computes D2/D3 as matmuls with scaled-identity stationary
    tiles accumulated in PSUM (psum = sum_j alpha_j * w_j), plus ~20
    bf16 warmup matmuls that lift the HAM clock gate before real work.
  - 32 accumulating f32r matmuls (4 bases x 8 K=128-chunks, N=512) into
    one PSUM bank -> out^T slice (128, 512) per core.

x and the cp_k^T slices are pre-permuted on host into SBUF layout so every
DMA is a plain contiguous (128, N) copy at full bandwidth:
  x:   [p, c*512 + b] = x[b, c*128 + p], split in two halves (c 0-3 / 4-7)
  w_k: [p, c*128 + o] = cp_k[o + 128*core, c*128 + p]

A general fallback path computes t = clip(|x|, 0, 1) explicitly. Path
choice only inspects the input range; both paths implement the full
reference function on device.
"""

import sys

if "/opt/trn_rl_repo" not in sys.path:
    sys.path.insert(0, "/opt/trn_rl_repo")

import numpy as np

import concourse.bacc as bacc
import concourse.mybir as mybir
from concourse.mybir import ActivationFunctionType as AF
from concourse.mybir import AluOpType as alu
from concourse.tile import TileContext
from concourse.bass_utils import run_bass_kernel_spmd

B = 512           # batch
I = 1024          # in_features
O = 1024          # out_features
NCORES = 8
OS = O // NCORES  # out_features per core = 128
CH = I // 128     # i-chunks of 128 = 8
HB = (CH // 2) * B  # x free-dim columns per half = 2048
WC = CH * OS      # weight free-dim columns = 1024

F32 = mybir.dt.float32
F32R = mybir.dt.float32r
SQ3 = 3.0 ** 0.5

_programs = {}


def _build(fast: bool):
    nc = bacc.Bacc("TRN2", target_bir_lowering=False, debug=False)
    xd = [
        nc.dram_tensor(f"x{h}", [128, HB], F32, kind="ExternalInput")
        for h in range(2)
    ]
    wd = [
        nc.dram_tensor(f"w{k}", [128, WC], F32, kind="ExternalInput")
        for k in range(4)
    ]
    outT = nc.dram_tensor("outT", [OS, B], F32, kind="ExternalOutput")

    with TileContext(nc) as tc:
        with (
            tc.tile_pool(name="p", bufs=1) as pool,
            tc.tile_pool(name="ps", bufs=1, space="PSUM") as pp,
        ):
            # scaled-identity stationary tiles, generated on device:
            # gpsimd writes f32 diag blocks, one ACT copy rounds to f32r
            CVALS = (1.0, -1.0, -2.0, 1.0 / 3.0, -1.0 / 3.0)
            craw = pool.tile([128, 5 * 128], F32, tag="craw", name="craw")
            nc.gpsimd.memset(craw[:], 0.0)
            for j, val in enumerate(CVALS):
                nc.gpsimd.affine_select(
                    out=craw[:, j * 128:(j + 1) * 128],
                    in_=craw[:, j * 128:(j + 1) * 128],
                    compare_op=alu.not_equal,
                    fill=val,
                    base=0,
                    pattern=[[-1, 128]],
                    channel_multiplier=1,
                )
            cblk = pool.tile([128, 5 * 128], F32R, tag="cblk", name="cblk")
            nc.scalar.copy(cblk[:], craw[:])
            consts = {
                nm: cblk[:, j * 128:(j + 1) * 128]
                for j, nm in enumerate(
                    ("cpos", "cneg", "cneg2", "cthird", "cnthird")
                )
            }

            # input DMAs, in arrival-priority order: xA, w0, w1, xB, w2, w3
            xs = [
                pool.tile([128, HB], F32R, tag=f"x{h}", name=f"x{h}")
                for h in range(2)
            ]
            w_sb = [
                pool.tile([128, WC], F32R, tag=f"w{k}", name=f"w{k}")
                for k in range(4)
            ]
            nc.sync.dma_start(out=xs[0][:], in_=xd[0].ap().bitcast(F32R))
            nc.sync.dma_start(out=w_sb[0][:], in_=wd[0].ap().bitcast(F32R))
            nc.sync.dma_start(out=w_sb[1][:], in_=wd[1].ap().bitcast(F32R))
            nc.sync.dma_start(out=xs[1][:], in_=xd[1].ap().bitcast(F32R))
            nc.sync.dma_start(out=w_sb[2][:], in_=wd[2].ap().bitcast(F32R))
            nc.sync.dma_start(out=w_sb[3][:], in_=wd[3].ap().bitcast(F32R))

            # x-side basis tensors, per half
            g1 = [pool.tile([128, HB], F32R, tag=f"g1{h}", name=f"g1{h}") for h in range(2)]
            g2 = [pool.tile([128, HB], F32R, tag=f"g2{h}", name=f"g2{h}") for h in range(2)]
            g3 = [pool.tile([128, HB], F32R, tag=f"g3{h}", name=f"g3{h}") for h in range(2)]
            if fast:
                # g1 = 3x^2, g3 = (g1/sqrt3)^2 = 3x^4, g2 = x*g1 = 3x^3
                nc.scalar.activation(g1[0][:], xs[0][:], AF.Square, scale=SQ3)
                nc.scalar.activation(g1[1][:], xs[1][:], AF.Square, scale=SQ3)
                nc.vector.tensor_mul(g2[0][:], xs[0][:], g1[0][:])
                nc.vector.tensor_mul(g2[1][:], xs[1][:], g1[1][:])
                nc.scalar.activation(g3[0][:], g1[0][:], AF.Square, scale=1.0 / SQ3)
                nc.scalar.activation(g3[1][:], g1[1][:], AF.Square, scale=1.0 / SQ3)
            else:
                for h in range(2):
                    ta = pool.tile([128, HB], F32, tag=f"ta{h}", name=f"ta{h}")
                    tt = pool.tile([128, HB], F32, tag=f"tt{h}", name=f"tt{h}")
                    t2 = pool.tile([128, HB], F32, tag=f"t2{h}", name=f"t2{h}")
                    # t = clip(|x|, 0, 1)
                    nc.scalar.activation(ta[:], xs[h][:], AF.Abs)
                    nc.vector.tensor_scalar(
                        tt[:], ta[:], 1.0, 0.0, alu.min, alu.max
                    )
                    nc.scalar.activation(t2[:], tt[:], AF.Square)
                    nc.vector.scalar_tensor_tensor(
                        g1[h][:], tt[:], 3.0, xs[h][:], alu.mult, alu.mult
                    )
                    nc.vector.scalar_tensor_tensor(
                        g2[h][:], t2[:], 3.0, xs[h][:], alu.mult, alu.mult
                    )
                    nc.vector.tensor_mul(g3[h][:], t2[:], g1[h][:])

            # transformed weights via TensorE: psum = sum_j alpha_j * w_j
            A_sb = pool.tile([128, WC], F32R, tag="A_sb", name="A_sb")
            D2_sb = pool.tile([128, WC], F32R, tag="D2_sb", name="D2_sb")
            D3_sb = pool.tile([128, WC], F32R, tag="D3_sb", name="D3_sb")
            ps_A = pp.tile([128, WC], F32, name="ps_A")
            ps_D2 = pp.tile([128, WC], F32, name="ps_D2")
            ps_D3 = pp.tile([128, WC], F32, name="ps_D3")
            psum = pp.tile([128, B], F32, name="psum")

            TRANSFORMS = [
                (ps_A, A_sb, [("cpos", 1), ("cneg", 0)]),
                (ps_D2, D2_sb, [("cpos", 0), ("cneg2", 1), ("cpos", 2)]),
                (ps_D3, D3_sb,
                 [("cthird", 3), ("cnthird", 0), ("cneg", 2), ("cpos", 1)]),
            ]

            def emit_transform_mms(ps, terms, h):
                sl = slice(h * 512, (h + 1) * 512)
                for i, (cn, k) in enumerate(terms):
                    nc.tensor.matmul(
                        ps[:, sl],
                        lhsT=consts[cn],
                        rhs=w_sb[k][:, sl],
                        start=(i == 0),
                        stop=(i == len(terms) - 1),
                    )

            G = [xs, g1, g2, g3]
            D = [w_sb[0], A_sb, D2_sb, D3_sb]

            mm_n = [0]

            def emit_main_wave(k, h):
                # 4 accumulating matmuls: bases k, x-half h (i-chunks 4h..4h+3)
                for c in range(4):
                    nc.tensor.matmul(
                        psum[:],
                        lhsT=D[k][:, (h * 4 + c) * OS:(h * 4 + c + 1) * OS],
                        rhs=G[k][h][:, c * B:(c + 1) * B],
                        start=(mm_n[0] == 0),
                        stop=(mm_n[0] == 31),
                    )
                    mm_n[0] += 1

            # PE warmup: idle bf16 matmuls on a memset scratch tile to lift
            # the HAM clock gate before real work arrives (results never
            # read; ps_A is cleared by the A transform's start=True later).
            # bf16 so it needs no f32r-rounded producer and starts at ~7us.
            wsc = pool.tile([128, 512], mybir.dt.bfloat16, tag="wsc", name="wsc")
            nc.gpsimd.memset(wsc[:], 1.0)
            for i in range(20):
                nc.tensor.matmul(
                    ps_A[:, 0:512],
                    lhsT=wsc[:, 0:128],
                    rhs=wsc[:],
                    start=(i == 0),
                    stop=(i == 19),
                )

            # PE program order ~ dependency readiness order
            emit_main_wave(0, 0)                       # needs xA, w0
            # A = w1 - w0 on DVE (idle window there; saves 4 PE matmuls)
            nc.vector.tensor_sub(A_sb[:], w_sb[1][:], w_sb[0][:])
            emit_main_wave(0, 1)                       # needs xB
            emit_main_wave(1, 0)                       # needs A_sb, g1A
            for h in range(2):                         # D2: needs w2
                emit_transform_mms(ps_D2, TRANSFORMS[1][2], h)
            emit_main_wave(1, 1)                       # needs g1B
            for h in range(2):                         # D3: needs w3
                emit_transform_mms(ps_D3, TRANSFORMS[2][2], h)
            nc.vector.tensor_copy(D2_sb[:], ps_D2[:])
            emit_main_wave(2, 0)                       # needs D2_sb, g2A
            for h in range(2):
                nc.vector.tensor_copy(
                    D3_sb[:, h * 512:(h + 1) * 512],
                    ps_D3[:, h * 512:(h + 1) * 512],
                )
            emit_main_wave(2, 1)                       # needs g2B
            emit_main_wave(3, 0)                       # needs D3_sb h0, g3A
            emit_main_wave(3, 1)                       # needs g3B

            osb = pool.tile([128, B], F32, tag="osb", name="osb")
            nc.vector.tensor_copy(osb[:], psum[:])  # DVE idle; faster PSUM read
            nc.sync.dma_start(out=outT.ap(), in_=osb[:])

    nc.compile()
    return nc


def _get_program(fast: bool):
    if fast not in _programs:
        _programs[fast] = _build(fast)
    return _programs[fast]


def _stage_x(x):
    # [p, c*512+b] = x[b, c*128+p]; split into halves (chunks 0-3 / 4-7)
    xt = x.T.reshape(CH, 128, B).transpose(1, 0, 2).reshape(128, CH * B)
    return (
        np.ascontiguousarray(xt[:, :HB]),
        np.ascontiguousarray(xt[:, HB:]),
    )


def _stage_w(cp, core):
    # [p, c*128+o] = cp[o + OS*core, c*128+p]
    sl = cp[core * OS:(core + 1) * OS].T  # (1024, 128) [i, o]
    return np.ascontiguousarray(
        sl.reshape(CH, 128, OS).transpose(1, 0, 2).reshape(128, WC)
    )


def make_in_maps(inputs):
    x = np.ascontiguousarray(np.asarray(inputs["x"], dtype=np.float32))
    cps = [
        np.ascontiguousarray(np.asarray(inputs[f"cp{k}"], dtype=np.float32))
        for k in range(4)
    ]
    xA, xB = _stage_x(x)
    in_maps = []
    for c in range(NCORES):
        m = {"x0": xA, "x1": xB}
        for k in range(4):
            m[f"w{k}"] = _stage_w(cps[k], c)
        in_maps.append(m)
    return in_maps


def kernel(**inputs) -> np.ndarray:
    x = np.asarray(inputs["x"], dtype=np.float32)
    fast = bool(x.min() >= 0.0) and bool(x.max() <= 1.0)
    nc = _get_program(fast)
    in_maps = make_in_maps(inputs)
    res = run_bass_kernel_spmd(nc, in_maps, core_ids=list(range(NCORES)))
    outT = np.concatenate(
        [res.results[c]["outT"] for c in range(NCORES)], axis=0
    )
    return np.ascontiguousarray(outT.T)



# revision 37
# speedup vs baseline: 2.9666x; 2.9666x over previous
"""BSplineSynapse Trainium2 kernel (8-core tensor-parallel over out_features).

Math: reference computes, with t = clip(|x|, 0, 1), s = 1 - t:
    w(t) = cp0*s^3 + 3*cp1*s^2*t + 3*cp2*s*t^2 + cp3*t^3   (per (o, i))
    out[b, o] = sum_i w[o, i](t[b, i]) * x[b, i]

Fast path (0 <= x <= 1, so t == x): w(x)*x is a quartic in x with zero
constant term. Rewritten in the CENTERED basis u = x - 1/2:
    out[b, o] = bias[o] + sum_j (u^j) @ E_j^T      (j = 1..4)
where E_j = recentred monomial weights and bias[o] = sum_i E_0[o, i].
The centered basis keeps both the moving tensors (|u^j| <= 2^-j) and the
weights small, so bf16 rounding stays ~3e-3 of max|out| (the naive
monomial basis at 0 gives ~1.5e-2 from cancellation amplification).

Per core (tensor-parallel over out_features, 128 rows each):
  - bf16 everywhere on the PE: 64 accumulating half-width matmuls
    (4 bases x 8 K=128-chunks x 2 batch halves, 256 moving cols); bias[o]
    is added for free during the PSUM drains (DVE tensor_scalar_add with
    a host-staged f32 per-partition column).
  - Inputs split over two parallel DMA queues (SP HWDGE + Pool SWDGE);
    u's first half arrives in octant slices so the PE starts ~1us in.
  - Elementwise: ACT does u2 = Square(u) (q0/q1), u4 = Square(u2); DVE
    does u2 q2/q3 (u*u) and u3 = u*u2, all quarter-granular to pipeline
    with the PE waves.
  - PE is kept continuously busy (a few warmup matmuls before the first
    octant lands) because the p-state ramp resets on idle gaps.
  - PSUM is split into two banks by batch half: bank A's matmuls finish
    first so its DVE drain + SP output DMA overlap the PE tail; bank B
    drains on ACT and goes out on the ACT HWDGE queue.

General path (any x): t = clip(|x|,0,1), u = t - 1/2 computed on host,
basis {x, u*x, u^2*x, u^3*x} with recentred cubic weights F_j; no bias.

Host staging only permutes/casts inputs and linearly recombines the cp
matrices (exact f64 math); all GEMM work runs on device.
"""

import sys

if "/opt/trn_rl_repo" not in sys.path:
    sys.path.insert(0, "/opt/trn_rl_repo")

from math import comb

import numpy as np
import ml_dtypes

import concourse.bacc as bacc
import concourse.mybir as mybir
from concourse.mybir import ActivationFunctionType as AF
from concourse.tile import TileContext
from concourse.bass_utils import run_bass_kernel_spmd

B = 512           # batch
I = 1024          # in_features
O = 1024          # out_features
NCORES = 8
OS = O // NCORES  # out_features per core = 128
CH = I // 128     # i-chunks of 128 = 8
W = CH * B        # staged x-side columns = 4096
WC = CH * OS      # staged weight columns = 1024

F32 = mybir.dt.float32
BF16 = mybir.dt.bfloat16
BF = ml_dtypes.bfloat16

N_WARM = 5        # 128-row warmup matmuls before the first real matmul

_programs = {}


def _build_fast():
    nc = bacc.Bacc("TRN2", target_bir_lowering=False, debug=False)
    ud = nc.dram_tensor("u", [128, W], BF16, kind="ExternalInput")
    ed = [
        nc.dram_tensor(f"e{k}", [128, WC], BF16, kind="ExternalInput")
        for k in range(1, 5)
    ]
    biasd = nc.dram_tensor("bias", [OS, 1], F32, kind="ExternalInput")
    outd = [
        nc.dram_tensor(f"out{h}", [OS, B // 2], F32, kind="ExternalOutput")
        for h in range(2)
    ]


    with TileContext(nc) as tc:
        with (
            tc.tile_pool(name="p", bufs=1) as pool,
            tc.tile_pool(name="ps", bufs=1, space="PSUM") as pp,
        ):
            u = pool.tile([128, W], BF16, tag="u", name="u")
            u2 = pool.tile([128, W], BF16, tag="u2", name="u2")
            u3 = pool.tile([128, W], BF16, tag="u3", name="u3")
            u4 = pool.tile([128, W], BF16, tag="u4", name="u4")
            e1a = pool.tile([128, 512], BF16, tag="e1a", name="e1a")
            e1b = pool.tile([128, 512], BF16, tag="e1b", name="e1b")
            e2 = pool.tile([128, WC], BF16, tag="e2", name="e2")
            e3 = pool.tile([128, WC], BF16, tag="e3", name="e3")
            e4 = pool.tile([128, WC], BF16, tag="e4", name="e4")
            biasc = pool.tile([OS, 1], F32, tag="biasc", name="biasc")
            wsc = pool.tile([128, 128], BF16, tag="wsc", name="wsc")
            scr = pool.tile([1, 1], BF16, tag="scr", name="scr")
            osb = [
                pool.tile([128, B // 2], F32, tag=f"osb{h}", name=f"osb{h}")
                for h in range(2)
            ]
            ps_w = pp.tile([128, 128], F32, name="ps_w")
            psa = pp.tile([128, B // 2], F32, name="psa")
            psb = pp.tile([128, B // 2], F32, name="psb")

            def oct_(t, j):       # octant view: i-chunk j, 512 batch cols
                return t[:, j * 512:(j + 1) * 512]

            def q_(t, q):         # quarter view: i-chunks 2q..2q+1
                return t[:, q * 1024:(q + 1) * 1024]

            # ---- DMA queue programs (parallel queues) ----
            # Each DMA has a ~500ns queue-slot floor, so only the first two
            # octants go individually (early PE start); later pairs merge.
            # SP: u oct0, oct1, oct2+3, E2, E3; out half A at the end
            nc.sync.dma_start(out=oct_(u, 0), in_=ud.ap()[:, 0:512])
            nc.sync.dma_start(out=oct_(u, 1), in_=ud.ap()[:, 512:1024])
            nc.sync.dma_start(out=oct_(u, 2), in_=ud.ap()[:, 1024:1536])
            nc.sync.dma_start(out=oct_(u, 3), in_=ud.ap()[:, 1536:2048])
            nc.sync.dma_start(out=e2[:], in_=ed[1].ap())
            nc.sync.dma_start(out=e3[:], in_=ed[2].ap())
            # Pool (SWDGE): E1a, u oct4+5, oct6+7, E1b, E4, bias. SWDGE
            # completion sems are ~1us slower than HWDGE, so nothing
            # latency-critical goes last here and outputs avoid this queue.
            nc.gpsimd.dma_start(out=e1a[:], in_=ed[0].ap()[:, 0:512])
            nc.gpsimd.dma_start(out=u[:, 2048:3072], in_=ud.ap()[:, 2048:3072])
            nc.gpsimd.dma_start(out=u[:, 3072:4096], in_=ud.ap()[:, 3072:4096])
            nc.gpsimd.dma_start(out=e1b[:], in_=ed[0].ap()[:, 512:1024])
            nc.gpsimd.dma_start(out=biasc[:], in_=biasd.ap())
            nc.gpsimd.dma_start(out=e4[:], in_=ed[3].ap())

            # ---- elementwise producers ----
            # (emission order: every reader AFTER its writer in trace order;
            # per-engine execution order is the per-engine subsequence)
            nc.vector.memset(wsc[:], 1.0)
            # ACT: tiny dummy Square first so the activation table (1283ns)
            # loads before real operands arrive (output to scratch so the
            # PE warmup's wsc reads don't serialize behind it)
            nc.scalar.activation(scr[0:1, 0:1], wsc[0:1, 0:1], AF.Square)
            # ACT: u2 q0/q1 = Square(u); DVE: u2 q2/q3 = u*u
            nc.scalar.activation(q_(u2, 0), q_(u, 0), AF.Square)
            nc.scalar.activation(q_(u2, 1), q_(u, 1), AF.Square)
            nc.vector.tensor_mul(q_(u2, 2), q_(u, 2), q_(u, 2))
            nc.vector.tensor_mul(q_(u2, 3), q_(u, 3), q_(u, 3))
            # DVE: u3 = u*u2 and u4 q3; ACT: u4 q0-q2 = Square(u2)
            for q in range(4):
                nc.vector.tensor_mul(q_(u3, q), q_(u, q), q_(u2, q))
            for q in range(3):
                nc.scalar.activation(q_(u4, q), q_(u2, q), AF.Square)
            nc.vector.tensor_mul(q_(u4, 3), q_(u2, 3), q_(u2, 3))

            # ---- PE program: warmup then gap-free accumulation waves ----
            # PSUM is split by batch half (psa: b 0-255, psb: b 256-511);
            # bank B finishes first so its drain + DMA hide under the
            # final bank-A matmuls.
            for i in range(N_WARM):
                nc.tensor.matmul(
                    ps_w[:], lhsT=wsc[:], rhs=wsc[:],
                    start=(i == 0), stop=(i == N_WARM - 1),
                )
            na = [0]
            nb = [0]
            NA = 32  # 32 half-width matmuls per PSUM bank (bias in drain)

            def half_mm(ps, cnt, lhsT, rhs):
                nc.tensor.matmul(
                    ps[:], lhsT=lhsT, rhs=rhs,
                    start=(cnt[0] == 0), stop=(cnt[0] == NA - 1),
                )
                cnt[0] += 1

            def wave_oct(et, echunk, g, j, half=None):
                lhsT = et[:, echunk * 128:(echunk + 1) * 128]
                c0 = j * 512
                if half in (None, 0):
                    half_mm(psa, na, lhsT, g[:, c0:c0 + 256])
                if half in (None, 1):
                    half_mm(psb, nb, lhsT, g[:, c0 + 256:c0 + 512])

            def wave_q(et, g, q, half=None):
                for c in (2 * q, 2 * q + 1):
                    wave_oct(et, c, g, c, half)

            # octant waves ordered by DMA arrival
            wave_oct(e1a, 0, u, 0)
            wave_oct(e1a, 1, u, 1)
            wave_oct(e1a, 2, u, 2)
            wave_oct(e1a, 3, u, 3)
            wave_oct(e1b, 0, u, 4)
            wave_oct(e1b, 1, u, 5)
            wave_oct(e1b, 2, u, 6)
            wave_q(e2, u2, 0)
            wave_oct(e1b, 3, u, 7)
            wave_q(e2, u2, 1)
            wave_q(e2, u2, 2)
            wave_q(e2, u2, 3)
            wave_q(e3, u3, 0)
            wave_q(e4, u4, 0)
            wave_q(e3, u3, 1)
            wave_q(e4, u4, 1)
            wave_q(e3, u3, 2)
            # tail: finish ALL of bank B first, then bank A. Bank B's ACT
            # drain + ACT-queue DMA (slower completion path) hide under the
            # remaining A matmuls; bank A's final drain goes out via the
            # faster SP completion path.
            wave_q(e3, u3, 3, half=1)
            wave_q(e4, u4, 2, half=1)
            wave_q(e4, u4, 3, half=1)
            # drain B emitted here: runs as soon as psb's stop fires;
            # bias[o] is added during the drain (f32, per-partition scalar)
            nc.vector.tensor_scalar_add(osb[1][:], psb[:], biasc[:])
            nc.scalar.dma_start(out=outd[1].ap(), in_=osb[1][:])
            wave_q(e3, u3, 3, half=0)
            wave_q(e4, u4, 2, half=0)
            wave_q(e4, u4, 3, half=0)
            assert na[0] == NA and nb[0] == NA

            nc.vector.tensor_scalar_add(osb[0][:], psa[:], biasc[:])
            nc.sync.dma_start(out=outd[0].ap(), in_=osb[0][:])

    nc.compile()
    return nc


def _build_general():
    nc = bacc.Bacc("TRN2", target_bir_lowering=False, debug=False)
    xd = nc.dram_tensor("x", [128, W], BF16, kind="ExternalInput")
    ud = nc.dram_tensor("u", [128, W], BF16, kind="ExternalInput")
    fd = [
        nc.dram_tensor(f"f{k}", [128, WC], BF16, kind="ExternalInput")
        for k in range(4)
    ]
    outd = [
        nc.dram_tensor(f"out{h}", [OS, B // 2], F32, kind="ExternalOutput")
        for h in range(2)
    ]

    with TileContext(nc) as tc:
        with (
            tc.tile_pool(name="p", bufs=1) as pool,
            tc.tile_pool(name="ps", bufs=1, space="PSUM") as pp,
        ):
            x = pool.tile([128, W], BF16, tag="x", name="x")
            u = pool.tile([128, W], BF16, tag="u", name="u")
            u2 = pool.tile([128, W], BF16, tag="u2", name="u2")
            m1 = pool.tile([128, W], BF16, tag="m1", name="m1")
            m2 = pool.tile([128, W], BF16, tag="m2", name="m2")
            m3 = pool.tile([128, W], BF16, tag="m3", name="m3")
            fs = [
                pool.tile([128, WC], BF16, tag=f"f{k}", name=f"f{k}")
                for k in range(4)
            ]
            wsc = pool.tile([128, 128], BF16, tag="wsc", name="wsc")
            osb = [
                pool.tile([128, B // 2], F32, tag=f"osb{h}", name=f"osb{h}")
                for h in range(2)
            ]
            ps_w = pp.tile([128, 128], F32, name="ps_w")
            psum = pp.tile([128, B], F32, name="psum")

            def oct_(t, j):
                return t[:, j * 512:(j + 1) * 512]

            def q_(t, q):
                return t[:, q * 1024:(q + 1) * 1024]

            # SP: x halves, f1, f3; Pool: f0, u halves, f2
            nc.sync.dma_start(out=x[:, 0:2048], in_=xd.ap()[:, 0:2048])
            nc.sync.dma_start(out=x[:, 2048:W], in_=xd.ap()[:, 2048:W])
            nc.sync.dma_start(out=fs[1][:], in_=fd[1].ap())
            nc.sync.dma_start(out=fs[3][:], in_=fd[3].ap())
            nc.gpsimd.dma_start(out=fs[0][:], in_=fd[0].ap())
            nc.gpsimd.dma_start(out=u[:, 0:2048], in_=ud.ap()[:, 0:2048])
            nc.gpsimd.dma_start(out=u[:, 2048:W], in_=ud.ap()[:, 2048:W])
            nc.gpsimd.dma_start(out=fs[2][:], in_=fd[2].ap())

            nc.vector.memset(wsc[:], 1.0)
            # DVE: m1 = u*x; m3 = u2*m1. ACT: u2 = Square(u); m2 = u2*x on DVE
            for q in range(4):
                nc.vector.tensor_mul(q_(m1, q), q_(u, q), q_(x, q))
            for q in range(4):
                nc.scalar.activation(q_(u2, q), q_(u, q), AF.Square)
            for q in range(4):
                nc.vector.tensor_mul(q_(m2, q), q_(u2, q), q_(x, q))
                nc.vector.tensor_mul(q_(m3, q), q_(u2, q), q_(m1, q))

            for i in range(N_WARM):
                nc.tensor.matmul(
                    ps_w[:], lhsT=wsc[:], rhs=wsc[:],
                    start=(i == 0), stop=(i == N_WARM - 1),
                )
            mm_n = [0]
            N_MM = 32

            def wave_oct(et, echunk, g, j):
                nc.tensor.matmul(
                    psum[:], lhsT=et[:, echunk * 128:(echunk + 1) * 128],
                    rhs=oct_(g, j),
                    start=(mm_n[0] == 0), stop=(mm_n[0] == N_MM - 1),
                )
                mm_n[0] += 1

            def wave_q(et, g, q):
                for c in (2 * q, 2 * q + 1):
                    wave_oct(et, c, g, c)

            for j in range(8):
                wave_oct(fs[0], j, x, j)
            for q in range(4):
                wave_q(fs[1], m1, q)
            for q in range(4):
                wave_q(fs[2], m2, q)
                wave_q(fs[3], m3, q)
            assert mm_n[0] == N_MM

            nc.vector.tensor_copy(osb[0][:], psum[:, 0:B // 2])
            nc.scalar.copy(osb[1][:], psum[:, B // 2:B])
            nc.sync.dma_start(out=outd[0].ap(), in_=osb[0][:])
            nc.gpsimd.dma_start(out=outd[1].ap(), in_=osb[1][:])

    nc.compile()
    return nc


def _get_program(fast: bool):
    if fast not in _programs:
        _programs[fast] = _build_fast() if fast else _build_general()
    return _programs[fast]


def _stage_xside(a):
    # [p, j*512 + b] = a[b, j*128 + p]  (f32/f64 in, bf16 out)
    st = a.T.reshape(CH, 128, B).transpose(1, 0, 2).reshape(128, W)
    return np.ascontiguousarray(st.astype(BF))


def _stage_w(wmat, core):
    # [p, c*128 + o] = wmat[o + 128*core, c*128 + p]
    sl = wmat[core * OS:(core + 1) * OS].T  # (1024, 128) [i, o]
    return np.ascontiguousarray(
        sl.reshape(CH, 128, OS).transpose(1, 0, 2).reshape(128, WC).astype(BF)
    )


def _weights(inputs, fast):
    cps = [np.asarray(inputs[f"cp{k}"], dtype=np.float64) for k in range(4)]
    # monomial (in t) coeffs of the cubic w(t)
    g = [
        cps[0],
        -3 * cps[0] + 3 * cps[1],
        3 * cps[0] - 6 * cps[1] + 3 * cps[2],
        -cps[0] + 3 * cps[1] - 3 * cps[2] + cps[3],
    ]
    if fast:
        # quartic p(x) = w(x)*x coeffs c_k (k=1..4 on x^k), recentred at 1/2
        c = [g[0], g[1], g[2], g[3]]  # c_{k+1} = g_k since p = w*x
        E = [
            sum(c[k - 1] * comb(k, j) * 0.5 ** (k - j) for k in range(max(j, 1), 5))
            for j in range(5)
        ]
        bias = E[0].sum(axis=1)  # (O,)
        return E[1:], bias
    else:
        # cubic w(t) recentred at 1/2: F_j, basis {x, ux, u^2 x, u^3 x}
        F = [
            sum(g[k] * comb(k, j) * 0.5 ** (k - j) for k in range(j, 4))
            for j in range(4)
        ]
        return F, None


def make_in_maps(inputs):
    x = np.asarray(inputs["x"], dtype=np.float64)
    fast = bool(x.min() >= 0.0) and bool(x.max() <= 1.0)
    if fast:
        E, bias = _weights(inputs, True)
        u_st = _stage_xside(x - 0.5)
        bias_f32 = bias.astype(np.float32)
        in_maps = []
        for c in range(NCORES):
            m = {"u": u_st}
            for k in range(4):
                m[f"e{k + 1}"] = _stage_w(E[k], c)
            m["bias"] = np.ascontiguousarray(
                bias_f32[c * OS:(c + 1) * OS].reshape(OS, 1)
            )
            in_maps.append(m)
        return in_maps
    else:
        F, _ = _weights(inputs, False)
        t = np.clip(np.abs(x), 0.0, 1.0)
        x_st = _stage_xside(x)
        u_st = _stage_xside(t - 0.5)
        in_maps = []
        for c in range(NCORES):
            m = {"x": x_st, "u": u_st}
            for k in range(4):
                m[f"f{k}"] = _stage_w(F[k], c)
            in_maps.append(m)
        return in_maps


def kernel(**inputs) -> np.ndarray:
    x = np.asarray(inputs["x"], dtype=np.float32)
    fast = bool(x.min() >= 0.0) and bool(x.max() <= 1.0)
    nc = _get_program(fast)
    in_maps = make_in_maps(inputs)
    res = run_bass_kernel_spmd(nc, in_maps, core_ids=list(range(NCORES)))
    out = np.empty((B, O), dtype=np.float32)
    for c in range(NCORES):
        sl = slice(c * OS, (c + 1) * OS)
        out[: B // 2, sl] = res.results[c]["out0"].T
        out[B // 2:, sl] = res.results[c]["out1"].T
    return out


# revision 39
# speedup vs baseline: 3.3065x; 1.1146x over previous
"""BSplineSynapse Trainium2 kernel (8-core tensor-parallel over out_features).

Math: reference computes, with t = clip(|x|, 0, 1), s = 1 - t:
    w(t) = cp0*s^3 + 3*cp1*s^2*t + 3*cp2*s*t^2 + cp3*t^3   (per (o, i))
    out[b, o] = sum_i w[o, i](t[b, i]) * x[b, i]

Fast path (0 <= x <= 1, so t == x): w(x)*x is a quartic in x with zero
constant term. Rewritten in the CENTERED basis u = x - 1/2:
    out[b, o] = bias[o] + sum_j (u^j) @ E_j^T      (j = 1..4)
where E_j = recentred monomial weights and bias[o] = sum_i E_0[o, i].
The centered basis keeps both the moving tensors (|u^j| <= 2^-j) and the
weights small, so bf16 rounding stays ~3e-3 of max|out| (the naive
monomial basis at 0 gives ~1.5e-2 from cancellation amplification).

Per core (tensor-parallel over out_features, 128 rows each):
  - bf16 everywhere on the PE: 64 accumulating half-width matmuls
    (4 bases x 8 K=128-chunks x 2 batch halves, 256 moving cols); bias[o]
    is added for free during the PSUM drains (DVE tensor_scalar_add with
    a host-staged f32 per-partition column).
  - Inputs split over two parallel DMA queues (SP HWDGE + Pool SWDGE);
    u's first half arrives in octant slices so the PE starts ~1us in.
  - Elementwise: ACT does u2 = Square(u) (q0/q1), u4 = Square(u2); DVE
    does u2 q2/q3 (u*u) and u3 = u*u2, all quarter-granular to pipeline
    with the PE waves.
  - PE is kept continuously busy (a few warmup matmuls before the first
    octant lands) because the p-state ramp resets on idle gaps.
  - PSUM is split into two banks by batch half: bank A's matmuls finish
    first so its DVE drain + SP output DMA overlap the PE tail; bank B
    drains on ACT and goes out on the ACT HWDGE queue.

General path (any x): t = clip(|x|,0,1), u = t - 1/2 computed on host,
basis {x, u*x, u^2*x, u^3*x} with recentred cubic weights F_j; no bias.

Host staging only permutes/casts inputs and linearly recombines the cp
matrices (exact f64 math); all GEMM work runs on device.
"""

import sys

if "/opt/trn_rl_repo" not in sys.path:
    sys.path.insert(0, "/opt/trn_rl_repo")

from math import comb

import numpy as np
import ml_dtypes

import concourse.bacc as bacc
import concourse.mybir as mybir
from concourse.mybir import ActivationFunctionType as AF
from concourse.tile import TileContext
from concourse.bass_utils import run_bass_kernel_spmd

B = 512           # batch
I = 1024          # in_features
O = 1024          # out_features
NCORES = 8
OS = O // NCORES  # out_features per core = 128
CH = I // 128     # i-chunks of 128 = 8
W = CH * B        # staged x-side columns = 4096
WC = CH * OS      # staged weight columns = 1024

F32 = mybir.dt.float32
BF16 = mybir.dt.bfloat16
FP8 = mybir.dt.float8e4
BF = ml_dtypes.bfloat16
F8 = ml_dtypes.float8_e4m3fn

N_WARM = 5        # 128-row warmup matmuls before the first real matmul

_programs = {}


def _build_fast():
    nc = bacc.Bacc("TRN2", target_bir_lowering=False, debug=False)
    ud = nc.dram_tensor("u", [128, W], BF16, kind="ExternalInput")
    ed = [
        nc.dram_tensor(f"e{k}", [128, WC], BF16, kind="ExternalInput")
        for k in range(1, 4)
    ]
    ed.append(nc.dram_tensor("e4", [128, WC], FP8, kind="ExternalInput"))
    biasd = nc.dram_tensor("bias", [OS, 1], F32, kind="ExternalInput")
    outd = [
        nc.dram_tensor(f"out{h}", [OS, B // 2], F32, kind="ExternalOutput")
        for h in range(2)
    ]


    with TileContext(nc) as tc:
        with (
            tc.tile_pool(name="p", bufs=1) as pool,
            tc.tile_pool(name="ps", bufs=1, space="PSUM") as pp,
        ):
            u = pool.tile([128, W], BF16, tag="u", name="u")
            u2 = pool.tile([128, W], BF16, tag="u2", name="u2")
            u3 = pool.tile([128, W], BF16, tag="u3", name="u3")
            u4 = pool.tile([128, CH, 512], FP8, tag="u4", name="u4")
            e1a = pool.tile([128, 512], BF16, tag="e1a", name="e1a")
            e1b = pool.tile([128, 512], BF16, tag="e1b", name="e1b")
            e2 = pool.tile([128, WC], BF16, tag="e2", name="e2")
            e3 = pool.tile([128, WC], BF16, tag="e3", name="e3")
            e4 = pool.tile([128, CH, 128], FP8, tag="e4", name="e4")
            biasc = pool.tile([OS, 1], F32, tag="biasc", name="biasc")
            wsc = pool.tile([128, 128], BF16, tag="wsc", name="wsc")
            scr = pool.tile([1, 1], BF16, tag="scr", name="scr")
            osb = [
                pool.tile([128, B // 2], F32, tag=f"osb{h}", name=f"osb{h}")
                for h in range(2)
            ]
            ps_w = pp.tile([128, 128], F32, name="ps_w")
            psa = pp.tile([128, B // 2], F32, name="psa")
            psb = pp.tile([128, B // 2], F32, name="psb")

            def oct_(t, j):       # octant view: i-chunk j, 512 batch cols
                return t[:, j * 512:(j + 1) * 512]

            def q_(t, q):         # quarter view: i-chunks 2q..2q+1
                return t[:, q * 1024:(q + 1) * 1024]

            # ---- DMA queue programs (parallel queues) ----
            # Each DMA has a ~500ns queue-slot floor, so only the first two
            # octants go individually (early PE start); later pairs merge.
            # SP: u oct0, oct1, oct2+3, E2, E3; out half A at the end
            nc.sync.dma_start(out=oct_(u, 0), in_=ud.ap()[:, 0:512])
            nc.sync.dma_start(out=oct_(u, 1), in_=ud.ap()[:, 512:1024])
            nc.sync.dma_start(out=oct_(u, 2), in_=ud.ap()[:, 1024:1536])
            nc.sync.dma_start(out=oct_(u, 3), in_=ud.ap()[:, 1536:2048])
            nc.sync.dma_start(out=e2[:], in_=ed[1].ap())
            nc.sync.dma_start(out=e3[:], in_=ed[2].ap())
            # Pool (SWDGE): E1a, u oct4+5, oct6+7, E1b, E4, bias. SWDGE
            # completion sems are ~1us slower than HWDGE, so nothing
            # latency-critical goes last here and outputs avoid this queue.
            nc.gpsimd.dma_start(out=e1a[:], in_=ed[0].ap()[:, 0:512])
            nc.gpsimd.dma_start(out=u[:, 2048:3072], in_=ud.ap()[:, 2048:3072])
            nc.gpsimd.dma_start(out=u[:, 3072:4096], in_=ud.ap()[:, 3072:4096])
            nc.gpsimd.dma_start(out=e1b[:], in_=ed[0].ap()[:, 512:1024])
            nc.gpsimd.dma_start(out=biasc[:], in_=biasd.ap())
            nc.gpsimd.dma_start(out=e4[:], in_=ed[3].ap())

            # ---- elementwise producers ----
            # (emission order: every reader AFTER its writer in trace order;
            # per-engine execution order is the per-engine subsequence)
            nc.vector.memset(wsc[:], 1.0)
            # ACT: tiny dummy Square first so the activation table (1283ns)
            # loads before real operands arrive (output to scratch so the
            # PE warmup's wsc reads don't serialize behind it)
            nc.scalar.activation(scr[0:1, 0:1], wsc[0:1, 0:1], AF.Square)
            # ACT: u2 q0/q1 = Square(u); DVE: u2 q2/q3 = u*u
            nc.scalar.activation(q_(u2, 0), q_(u, 0), AF.Square)
            nc.scalar.activation(q_(u2, 1), q_(u, 1), AF.Square)
            nc.vector.tensor_mul(q_(u2, 2), q_(u, 2), q_(u, 2))
            nc.vector.tensor_mul(q_(u2, 3), q_(u, 3), q_(u, 3))
            for q in range(3):
                nc.vector.tensor_mul(q_(u3, q), q_(u, q), q_(u2, q))
            nc.gpsimd.tensor_mul(q_(u3, 3), q_(u, 3), q_(u2, 3))
            for q in range(3):
                nc.scalar.activation(u4[:, 2 * q:2 * q + 2, :], q_(u2, q), AF.Square)
            nc.gpsimd.tensor_mul(u4[:, 6:8, :], q_(u2, 3), q_(u2, 3))

            # ---- PE program: warmup then gap-free accumulation waves ----
            # PSUM is split by batch half (psa: b 0-255, psb: b 256-511);
            # bank B finishes first so its drain + DMA hide under the
            # final bank-A matmuls.
            for i in range(N_WARM):
                nc.tensor.matmul(
                    ps_w[:], lhsT=wsc[:], rhs=wsc[:],
                    start=(i == 0), stop=(i == N_WARM - 1),
                )
            na = [0]
            nb = [0]
            NA = 28  # 24 bf16 + 4 fp8-DoubleRow matmuls per bank

            def half_mm(ps, cnt, lhsT, rhs):
                nc.tensor.matmul(
                    ps[:], lhsT=lhsT, rhs=rhs,
                    start=(cnt[0] == 0), stop=(cnt[0] == NA - 1),
                )
                cnt[0] += 1

            def wave_oct(et, echunk, g, j, half=None):
                lhsT = et[:, echunk * 128:(echunk + 1) * 128]
                c0 = j * 512
                if half in (None, 0):
                    half_mm(psa, na, lhsT, g[:, c0:c0 + 256])
                if half in (None, 1):
                    half_mm(psb, nb, lhsT, g[:, c0 + 256:c0 + 512])

            def wave_q(et, g, q, half=None):
                for c in (2 * q, 2 * q + 1):
                    wave_oct(et, c, g, c, half)

            def wave_dr(cp, half=None):
                lhsT = e4[:, 2 * cp:2 * cp + 2, :]
                if half in (None, 0):
                    nc.tensor.matmul(
                        psa[:], lhsT=lhsT, rhs=u4[:, 2 * cp:2 * cp + 2, 0:256],
                        start=(na[0] == 0), stop=(na[0] == NA - 1),
                        perf_mode=mybir.MatmulPerfMode.DoubleRow,
                    )
                    na[0] += 1
                if half in (None, 1):
                    nc.tensor.matmul(
                        psb[:], lhsT=lhsT, rhs=u4[:, 2 * cp:2 * cp + 2, 256:512],
                        start=(nb[0] == 0), stop=(nb[0] == NA - 1),
                        perf_mode=mybir.MatmulPerfMode.DoubleRow,
                    )
                    nb[0] += 1

            # octant waves ordered by DMA arrival
            wave_oct(e1a, 0, u, 0)
            wave_oct(e1a, 1, u, 1)
            wave_oct(e1a, 2, u, 2)
            wave_oct(e1a, 3, u, 3)
            wave_oct(e1b, 0, u, 4)
            wave_oct(e1b, 1, u, 5)
            wave_oct(e1b, 2, u, 6)
            wave_q(e2, u2, 0)
            wave_oct(e1b, 3, u, 7)
            wave_q(e2, u2, 1)
            wave_q(e2, u2, 2)
            wave_q(e2, u2, 3)
            wave_q(e3, u3, 0)
            wave_dr(0)
            wave_q(e3, u3, 1)
            wave_dr(1)
            wave_q(e3, u3, 2)
            # tail: finish ALL of bank B first, then bank A. Bank B's ACT
            # drain + ACT-queue DMA (slower completion path) hide under the
            # remaining A matmuls; bank A's final drain goes out via the
            # faster SP completion path.
            wave_q(e3, u3, 3, half=1)
            wave_dr(2, half=1)
            wave_dr(3, half=1)
            # drain B emitted here: runs as soon as psb's stop fires;
            # bias[o] is added during the drain (f32, per-partition scalar)
            nc.vector.tensor_scalar_add(osb[1][:], psb[:], biasc[:])
            nc.scalar.dma_start(out=outd[1].ap(), in_=osb[1][:])
            wave_q(e3, u3, 3, half=0)
            wave_dr(2, half=0)
            wave_dr(3, half=0)
            assert na[0] == NA and nb[0] == NA

            nc.vector.tensor_scalar_add(osb[0][:], psa[:], biasc[:])
            nc.sync.dma_start(out=outd[0].ap(), in_=osb[0][:])

    nc.compile()
    return nc


def _build_general():
    nc = bacc.Bacc("TRN2", target_bir_lowering=False, debug=False)
    xd = nc.dram_tensor("x", [128, W], BF16, kind="ExternalInput")
    ud = nc.dram_tensor("u", [128, W], BF16, kind="ExternalInput")
    fd = [
        nc.dram_tensor(f"f{k}", [128, WC], BF16, kind="ExternalInput")
        for k in range(4)
    ]
    outd = [
        nc.dram_tensor(f"out{h}", [OS, B // 2], F32, kind="ExternalOutput")
        for h in range(2)
    ]

    with TileContext(nc) as tc:
        with (
            tc.tile_pool(name="p", bufs=1) as pool,
            tc.tile_pool(name="ps", bufs=1, space="PSUM") as pp,
        ):
            x = pool.tile([128, W], BF16, tag="x", name="x")
            u = pool.tile([128, W], BF16, tag="u", name="u")
            u2 = pool.tile([128, W], BF16, tag="u2", name="u2")
            m1 = pool.tile([128, W], BF16, tag="m1", name="m1")
            m2 = pool.tile([128, W], BF16, tag="m2", name="m2")
            m3 = pool.tile([128, W], BF16, tag="m3", name="m3")
            fs = [
                pool.tile([128, WC], BF16, tag=f"f{k}", name=f"f{k}")
                for k in range(4)
            ]
            wsc = pool.tile([128, 128], BF16, tag="wsc", name="wsc")
            osb = [
                pool.tile([128, B // 2], F32, tag=f"osb{h}", name=f"osb{h}")
                for h in range(2)
            ]
            ps_w = pp.tile([128, 128], F32, name="ps_w")
            psum = pp.tile([128, B], F32, name="psum")

            def oct_(t, j):
                return t[:, j * 512:(j + 1) * 512]

            def q_(t, q):
                return t[:, q * 1024:(q + 1) * 1024]

            # SP: x halves, f1, f3; Pool: f0, u halves, f2
            nc.sync.dma_start(out=x[:, 0:2048], in_=xd.ap()[:, 0:2048])
            nc.sync.dma_start(out=x[:, 2048:W], in_=xd.ap()[:, 2048:W])
            nc.sync.dma_start(out=fs[1][:], in_=fd[1].ap())
            nc.sync.dma_start(out=fs[3][:], in_=fd[3].ap())
            nc.gpsimd.dma_start(out=fs[0][:], in_=fd[0].ap())
            nc.gpsimd.dma_start(out=u[:, 0:2048], in_=ud.ap()[:, 0:2048])
            nc.gpsimd.dma_start(out=u[:, 2048:W], in_=ud.ap()[:, 2048:W])
            nc.gpsimd.dma_start(out=fs[2][:], in_=fd[2].ap())

            nc.vector.memset(wsc[:], 1.0)
            # DVE: m1 = u*x; m3 = u2*m1. ACT: u2 = Square(u); m2 = u2*x on DVE
            for q in range(4):
                nc.vector.tensor_mul(q_(m1, q), q_(u, q), q_(x, q))
            for q in range(4):
                nc.scalar.activation(q_(u2, q), q_(u, q), AF.Square)
            for q in range(4):
                nc.vector.tensor_mul(q_(m2, q), q_(u2, q), q_(x, q))
                nc.vector.tensor_mul(q_(m3, q), q_(u2, q), q_(m1, q))

            for i in range(N_WARM):
                nc.tensor.matmul(
                    ps_w[:], lhsT=wsc[:], rhs=wsc[:],
                    start=(i == 0), stop=(i == N_WARM - 1),
                )
            mm_n = [0]
            N_MM = 32

            def wave_oct(et, echunk, g, j):
                nc.tensor.matmul(
                    psum[:], lhsT=et[:, echunk * 128:(echunk + 1) * 128],
                    rhs=oct_(g, j),
                    start=(mm_n[0] == 0), stop=(mm_n[0] == N_MM - 1),
                )
                mm_n[0] += 1

            def wave_q(et, g, q):
                for c in (2 * q, 2 * q + 1):
                    wave_oct(et, c, g, c)

            for j in range(8):
                wave_oct(fs[0], j, x, j)
            for q in range(4):
                wave_q(fs[1], m1, q)
            for q in range(4):
                wave_q(fs[2], m2, q)
                wave_q(fs[3], m3, q)
            assert mm_n[0] == N_MM

            nc.vector.tensor_copy(osb[0][:], psum[:, 0:B // 2])
            nc.scalar.copy(osb[1][:], psum[:, B // 2:B])
            nc.sync.dma_start(out=outd[0].ap(), in_=osb[0][:])
            nc.gpsimd.dma_start(out=outd[1].ap(), in_=osb[1][:])

    nc.compile()
    return nc


def _get_program(fast: bool):
    if fast not in _programs:
        _programs[fast] = _build_fast() if fast else _build_general()
    return _programs[fast]


def _stage_xside(a):
    # [p, j*512 + b] = a[b, j*128 + p]  (f32/f64 in, bf16 out)
    st = a.T.reshape(CH, 128, B).transpose(1, 0, 2).reshape(128, W)
    return np.ascontiguousarray(st.astype(BF))


def _stage_w(wmat, core):
    # [p, c*128 + o] = wmat[o + 128*core, c*128 + p]
    sl = wmat[core * OS:(core + 1) * OS].T  # (1024, 128) [i, o]
    return np.ascontiguousarray(
        sl.reshape(CH, 128, OS).transpose(1, 0, 2).reshape(128, WC).astype(BF)
    )


def _weights(inputs, fast):
    cps = [np.asarray(inputs[f"cp{k}"], dtype=np.float64) for k in range(4)]
    # monomial (in t) coeffs of the cubic w(t)
    g = [
        cps[0],
        -3 * cps[0] + 3 * cps[1],
        3 * cps[0] - 6 * cps[1] + 3 * cps[2],
        -cps[0] + 3 * cps[1] - 3 * cps[2] + cps[3],
    ]
    if fast:
        # quartic p(x) = w(x)*x coeffs c_k (k=1..4 on x^k), recentred at 1/2
        c = [g[0], g[1], g[2], g[3]]  # c_{k+1} = g_k since p = w*x
        E = [
            sum(c[k - 1] * comb(k, j) * 0.5 ** (k - j) for k in range(max(j, 1), 5))
            for j in range(5)
        ]
        bias = E[0].sum(axis=1)  # (O,)
        return E[1:], bias
    else:
        # cubic w(t) recentred at 1/2: F_j, basis {x, ux, u^2 x, u^3 x}
        F = [
            sum(g[k] * comb(k, j) * 0.5 ** (k - j) for k in range(j, 4))
            for j in range(4)
        ]
        return F, None


def make_in_maps(inputs):
    x = np.asarray(inputs["x"], dtype=np.float64)
    fast = bool(x.min() >= 0.0) and bool(x.max() <= 1.0)
    if fast:
        E, bias = _weights(inputs, True)
        u_st = _stage_xside(x - 0.5)
        bias_f32 = bias.astype(np.float32)
        in_maps = []
        for c in range(NCORES):
            m = {"u": u_st}
            for k in range(3):
                m[f"e{k + 1}"] = _stage_w(E[k], c)
            m["e4"] = np.ascontiguousarray(
                _stage_w(E[3], c).astype(np.float32).astype(F8)
            )
            m["bias"] = np.ascontiguousarray(
                bias_f32[c * OS:(c + 1) * OS].reshape(OS, 1)
            )
            in_maps.append(m)
        return in_maps
    else:
        F, _ = _weights(inputs, False)
        t = np.clip(np.abs(x), 0.0, 1.0)
        x_st = _stage_xside(x)
        u_st = _stage_xside(t - 0.5)
        in_maps = []
        for c in range(NCORES):
            m = {"x": x_st, "u": u_st}
            for k in range(4):
                m[f"f{k}"] = _stage_w(F[k], c)
            in_maps.append(m)
        return in_maps


def kernel(**inputs) -> np.ndarray:
    x = np.asarray(inputs["x"], dtype=np.float32)
    fast = bool(x.min() >= 0.0) and bool(x.max() <= 1.0)
    nc = _get_program(fast)
    in_maps = make_in_maps(inputs)
    res = run_bass_kernel_spmd(nc, in_maps, core_ids=list(range(NCORES)))
    out = np.empty((B, O), dtype=np.float32)
    for c in range(NCORES):
        sl = slice(c * OS, (c + 1) * OS)
        out[: B // 2, sl] = res.results[c]["out0"].T
        out[B // 2:, sl] = res.results[c]["out1"].T
    return out
